# revision 1
# baseline (speedup 1.0000x reference)
"""Trainium2 Bass kernel for nn_AttentionLayer_60894046322746.

Full attention layer: fused QKV projection + (elementwise) rotary + softmax
attention with additive bias + output projection.

  B=2, S=2048, HID=1024, NH=16, DH=64, ROT=32, fp32 inputs/outputs.

Sharding: 8 cores = 2 batch groups x 4 query shards; NO collectives.
Core i handles batch b=i//4, query rows [512*(i%4), 512*(i%4+1)).
Each core computes K^T and V for its batch's FULL 2048 tokens (redundant
across the 4 cores of a batch group — far cheaper than an AllGather at
the interconnect's effective bandwidth), plus Q for its own 512 queries,
then attention + projection for its query slice. Host concatenates the
8 [512, 1024] output slices.

SPMD trick: all cores run one program; the host ROTATES the token axis
per core (np.roll) so each core's queries sit at tokens [0, 512) of its
own xT; K/V/bias columns follow the same rotation (softmax sums over k
in any order).

Device compute (matmul out = lhsT.T @ rhs, contraction over partitions):
  Q,K: fp8e4 DoubleRow matmuls (K=256/instr, 0.5 cyc/row), single pass —
    x,W host-quantized, W prescaled 2^6 (e4m3 subnormal avoidance);
    dequant+rotary+q-scale folded into the PSUM->SBUF map multiply (DVE).
  V: fp8e4 DoubleRow, THREE passes (x8@W8 + rx8@W8 + x8@rW8) where
    rx8/rW8 are host-side quantization residuals — recovers ~bf16
    accuracy at 3/8 the bf16 matmul cost. PSUM->SBUF copy writes bf16
    V' tiles with a per-head ones column (memset).
  scores S^T[k,q] = K_tile.T @ Q_head   (bf16, K=64)
  E0 = exp(S) on ACT (PSUM -> SBUF bf16, 2 k-tiles per instruction)
  E = E0 * exp(bias)^T                  (exp(bias) host-precomputed bf16;
    elementwise on DVE (2x mode) / Pool (SBUF-only engines; the real HW
    Pool engine cannot read PSUM, so the bias cannot be added pre-exp)
  ctx'^T[65,q] += V'_h[kt].T @ E_h[kt]  (bf16; ones col -> row 64 holds
    the softmax denominator)
  normalize: reciprocal + partition_broadcast + DVE mul -> ctxpair bf16
  out[q,m] = ctxpair.T @ projW          (bf16)
"""
import os
import sys
import time

for _p in ("/opt/trn_rl_repo", "/root/.axon_site/_ro/trn_rl_repo"):
    if os.path.isdir(_p) and _p not in sys.path:
        sys.path.insert(0, _p)

import numpy as np
import ml_dtypes

from concourse import bass, bacc, tile, mybir
from concourse.bass_utils import run_bass_kernel_spmd

F32 = mybir.dt.float32
BF16 = mybir.dt.bfloat16
FP8 = mybir.dt.float8e4
AF = mybir.ActivationFunctionType
bf16 = ml_dtypes.bfloat16
f8 = ml_dtypes.float8_e4m3

B, S, HID = 2, 2048, 1024
DH, NH, ROT = 64, 16, 32
SQ = S // 4            # queries per core
NKT = S // 128         # 16 k-token tiles
NPAIR = NH // 2        # 8 head pairs
N_CORES = 8
WSCALE = 64.0          # fp8 weight prescale (2^6)

_CACHED_NC = None


def _build_nc(dbg=False):
    nc = bacc.Bacc("TRN2", target_bir_lowering=False, debug=False,
                   num_devices=N_CORES)

    # ---- per-core DRAM parameters (host-prepared shards) ----
    xT_d = nc.dram_tensor("xT", [HID, S], FP8, kind="ExternalInput")
    rxT_d = nc.dram_tensor("rxT", [HID, S], FP8, kind="ExternalInput")
    wqk_d = nc.dram_tensor("wqk", [HID, 2048], FP8, kind="ExternalInput")
    wv_d = nc.dram_tensor("wv", [HID, HID], FP8, kind="ExternalInput")
    rwv_d = nc.dram_tensor("rwv", [HID, HID], FP8, kind="ExternalInput")
    biasT_d = nc.dram_tensor("biasT", [S, SQ], BF16, kind="ExternalInput")
    mq_d = nc.dram_tensor("mq", [128, SQ], BF16, kind="ExternalInput")
    mk_d = nc.dram_tensor("mk", [128, S], BF16, kind="ExternalInput")
    projw_d = nc.dram_tensor("projw", [HID, HID], BF16, kind="ExternalInput")
    out_d = nc.dram_tensor("out", [SQ, HID], F32, kind="ExternalOutput")

    dbg_d = {}
    if dbg:
        for nm, shp, dt_ in [
            ("dbg_q", [128, SQ], F32), ("dbg_k", [128, S], F32),
            ("dbg_v", [128, NH * 65], F32), ("dbg_e", [128, 4 * SQ], F32),
            ("dbg_ctx", [65, SQ], F32),
        ]:
            dbg_d[nm] = nc.dram_tensor(nm, shp, dt_, kind="ExternalOutput")

    with tile.TileContext(nc) as tc:
        _build_body(nc, tc, xT_d, rxT_d, wqk_d, wv_d, rwv_d, biasT_d,
                    mq_d, mk_d, projw_d, out_d, dbg_d)
    nc.compile()
    return nc


def _build_body(nc, tc, xT_d, rxT_d, wqk_d, wv_d, rwv_d, biasT_d,
                mq_d, mk_d, projw_d, out_d, dbg_d=None):
    dbg_d = dbg_d or {}
    DR = mybir.MatmulPerfMode.DoubleRow

    with (
        tc.tile_pool(name="persist", bufs=1) as pp,
        tc.tile_pool(name="dram", bufs=1, space="DRAM") as dp,
    ):
        # persistent SBUF (per-partition KiB in comments)
        xT_sb = pp.tile([128, 8, S], FP8, name="xT_sb")            # 16
        rxT_sb = pp.tile([128, 8, S], FP8, name="rxT_sb")          # 16
        k_sb = pp.tile([128, NPAIR, S], BF16, name="k_sb")         # 32
        q_sb = pp.tile([128, NPAIR, SQ], BF16, name="q_sb")        # 8
        v_sb = pp.tile([128, NKT, NH * 65], BF16, name="v_sb")     # 33.3
        biasT_sb = pp.tile([128, NKT, SQ], BF16, name="biasT_sb")  # 16
        mq_sb = pp.tile([128, SQ], BF16, name="mq_sb")             # 1
        mk_sb = pp.tile([128, S], BF16, name="mk_sb")              # 4
        projw_sb = pp.tile([128, 8, HID], BF16, name="projw_sb")   # 16
        ctxpair_sb = pp.tile([128, NPAIR, SQ], BF16, name="ctxpair_sb")

        # DMA issue order = DMA device service order: first the tensors
        # the first few PE instructions need, big late-need tensors later.
        def dma_xt(j):
            # token-range split: chunk j = tokens [512j, 512j+512) across
            # ALL k-chunks, so chunk 0 alone unblocks every Q matmul and
            # the first K token-chunk
            nc.sync.dma_start(
                out=xT_sb[:, :, 512 * j:512 * (j + 1)],
                in_=xT_d[:, 512 * j:512 * (j + 1)]
                .rearrange("(c p) t -> p c t", p=128))

        def dma_bias(j):
            nc.sync.dma_start(
                out=biasT_sb[:, 4 * j:4 * (j + 1), :],
                in_=biasT_d[512 * j:512 * (j + 1), :]
                .rearrange("(kt p) q -> p kt q", p=128))

        with (
            tc.tile_pool(name="qkv_w", bufs=2) as wp,
            tc.tile_pool(name="qkv_ps", bufs=2, space="PSUM") as qps,
            tc.tile_pool(name="att_sps", bufs=2, space="PSUM") as sps,
            tc.tile_pool(name="att_e", bufs=14) as ep,
            tc.tile_pool(name="att_cps", bufs=2, space="PSUM") as cps,
            tc.tile_pool(name="att_norm", bufs=1) as npo,
        ):
            def dma_wt(p):
                wt = wp.tile([128, 8, 256], FP8, tag="wqk", name="wt")
                nc.sync.dma_start(
                    out=wt[:, :, 128:256],
                    in_=wqk_d[:, 128 * p:128 * (p + 1)]
                    .rearrange("(c pp_) m -> pp_ c m", pp_=128))
                nc.sync.dma_start(
                    out=wt[:, :, 0:128],
                    in_=wqk_d[:, 1024 + 128 * p:1024 + 128 * (p + 1)]
                    .rearrange("(c pp_) m -> pp_ c m", pp_=128))
                return wt

            def kq_pair(p, wt):
                """Q dims (own 512) then K dims (full 2048 tokens) for
                head pair p -> q_sb/k_sb; rotary+dequant via mq/mk maps.
                Q first so the pair's first score tiles unblock sooner."""
                ps = qps.tile([128, 512], F32, tag="qkvps", name="q_ps")
                for j in range(4):
                    nc.tensor.matmul(
                        ps[:], wt[:, 2 * j:2 * j + 2, 128:256],
                        xT_sb[:, 2 * j:2 * j + 2, 0:512],
                        start=(j == 0), stop=(j == 3), perf_mode=DR)
                nc.vector.tensor_mul(q_sb[:, p, :], ps[:], mq_sb[:])
                for tch in range(4):
                    ps = qps.tile([128, 512], F32, tag="qkvps", name="k_ps")
                    for j in range(4):
                        nc.tensor.matmul(
                            ps[:], wt[:, 2 * j:2 * j + 2, 0:128],
                            xT_sb[:, 2 * j:2 * j + 2,
                                  512 * tch:512 * (tch + 1)],
                            start=(j == 0), stop=(j == 3), perf_mode=DR)
                    nc.vector.tensor_mul(
                        k_sb[:, p, 512 * tch:512 * (tch + 1)], ps[:],
                        mk_sb[:, 512 * tch:512 * (tch + 1)])

            def dma_wv(c):
                w8c = wp.tile([128, 8, 512], FP8, tag="wv8", bufs=1, name="w8c")
                rwc = wp.tile([128, 8, 512], FP8, tag="rwv", bufs=1, name="rwc")
                nc.sync.dma_start(
                    out=w8c[:],
                    in_=wv_d[:, 512 * c:512 * (c + 1)]
                    .rearrange("(cc pp_) m -> pp_ cc m", pp_=128))
                nc.sync.dma_start(
                    out=rwc[:],
                    in_=rwv_d[:, 512 * c:512 * (c + 1)]
                    .rearrange("(cc pp_) m -> pp_ cc m", pp_=128))
                return w8c, rwc

            def v_chunk(c, w8c, rwc, tts):
                """V token-major for 8 heads (c in {0,1}), 3-pass residual-
                compensated fp8: x8@W8 + x8@rW8 + rx8@W8 (x-residual last
                so its DMA arrival overlaps the first two passes)."""
                for tt in tts:
                    ps = qps.tile([128, 512], F32, tag="qkvps", name="v_ps")
                    for pi, (xt, wt_) in enumerate(
                            ((xT_sb, w8c), (xT_sb, rwc), (rxT_sb, w8c))):
                        for j in range(4):
                            nc.tensor.matmul(
                                ps[:],
                                xt[:, 2 * j:2 * j + 2,
                                   128 * tt:128 * (tt + 1)],
                                wt_[:, 2 * j:2 * j + 2, :],
                                start=(pi == 0 and j == 0),
                                stop=(pi == 2 and j == 3), perf_mode=DR)
                    vslot = (v_sb[:, tt, 65 * 8 * c:65 * 8 * (c + 1)]
                             .rearrange("pp_ (h d) -> pp_ h d", h=8))
                    nc.vector.tensor_scalar_mul(
                        vslot[:, :, 0:64],
                        ps[:].rearrange("pp_ (h d) -> pp_ h d", h=8),
                        1.0 / WSCALE)

            def scores_head(h):
                """scores -> exp -> *exp(bias) for head h; returns 8 E
                tiles [128, 2, SQ] bf16 (each covering 2 k-tiles)."""
                p, hi = h // 2, h % 2
                base = 64 * hi
                es = []
                for J in range(8):
                    e_t = ep.tile([128, 2, SQ], BF16, tag="e", name="e")
                    st = sps.tile([128, 2, SQ], F32, tag="st", name="st")
                    for kk in range(2):
                        kt = 2 * J + kk
                        nc.tensor.matmul(
                            st[:, kk, :],
                            k_sb[base:base + 64, p,
                                 128 * kt:128 * (kt + 1)],
                            q_sb[base:base + 64, p, :],
                            start=True, stop=True)
                    nc.scalar.activation(e_t[:], st[:], AF.Exp)
                    # bias multiply (exp(bias), host-precomputed); Pool
                    # cannot read PSUM so this runs post-exp in SBUF.
                    # All on Pool: it has slack, and keeping DVE free
                    # shortens the tail normalize chains.
                    nc.gpsimd.tensor_mul(e_t[:], e_t[:],
                                         biasT_sb[:, 2 * J:2 * J + 2, :])
                    es.append(e_t)
                return es

            def pv_head(h, es):
                """PV (bf16, ones col -> denominator) + normalize into
                ctxpair_sb for head h."""
                p, hi = h // 2, h % 2
                ctx = cps.tile([65, SQ], F32, tag="ctx", name="ctx")
                for kt in range(NKT):
                    nc.tensor.matmul(
                        ctx[:],
                        v_sb[:, kt, 65 * h:65 * (h + 1)],
                        es[kt // 2][:, kt % 2, :],
                        start=(kt == 0), stop=(kt == NKT - 1))
                if h == 0 and "dbg_ctx" in dbg_d:
                    cd = npo.tile([65, SQ], F32, tag="cd", name="cd")
                    nc.vector.tensor_copy(cd[:], ctx[:])
                    nc.sync.dma_start(out=dbg_d["dbg_ctx"][:], in_=cd[:])
                # reciprocal of the denominator row, then broadcast it to
                # partitions 0..63 with three 32-partition stream_shuffles
                # (all on DVE — no DRAM bounce / partition_broadcast)
                s_sb = npo.tile([128, SQ], F32, tag="s", bufs=2,
                                name="s_sb")
                # memset claims rows 65..95 for this tile generation (the
                # shuffle below reads the whole quadrant); runs during PV
                nc.vector.memset(s_sb[64:96, :], 0.0)
                nc.vector.reciprocal(s_sb[64:65, :], ctx[64:65, :])
                bc = [0] * 32
                nc.vector.stream_shuffle(s_sb[96:128, :], s_sb[64:96, :], bc)
                nc.vector.stream_shuffle(s_sb[32:64, :], s_sb[96:128, :], bc)
                nc.vector.stream_shuffle(s_sb[0:32, :], s_sb[96:128, :], bc)
                nc.vector.tensor_mul(
                    ctxpair_sb[64 * hi:64 * (hi + 1), p, :],
                    ctx[0:64, :], s_sb[0:64, :])

            # -------- issue order: feed ACT early; PV lags scores by --------
            # -------- <=3 heads so the e/sp/st rings never deadlock  --------
            # DMA order: pair-0 weights + first x/bias chunks first, the
            # big late-need tensors (rxT, projw) behind them.
            # all 256 per-head ones columns of V' in one strided memset
            # (keeps the per-token-tile V copy chain one op shorter)
            nc.vector.memset(
                v_sb[:].rearrange("pp_ t (h d) -> pp_ t h d", h=NH)
                [:, :, :, 64:65], 1.0)
            dma_xt(0)
            wt0 = dma_wt(0)
            dma_xt(1)
            dma_xt(2)
            dma_xt(3)
            nc.sync.dma_start(out=mq_sb[:], in_=mq_d[:])
            nc.sync.dma_start(out=mk_sb[:], in_=mk_d[:])
            dma_bias(0)
            wv0 = dma_wv(0)
            # rxT gates PE (V pass 3); bias chunks 1-3 and wt1 only feed
            # slack engines / later blocks, so they queue after it
            nc.sync.dma_start(
                out=rxT_sb[:],
                in_=rxT_d[:].rearrange("(c p) t -> p c t", p=128))
            wt1 = dma_wt(1)
            dma_bias(1)
            dma_bias(2)
            dma_bias(3)

            es_of = {}
            kq_pair(0, wt0)
            es_of[0] = scores_head(0)
            es_of[1] = scores_head(1)
            v_chunk(0, *wv0, tts=range(0, 8))
            nexth = 0
            wts = {1: wt1}
            wv1 = None
            for p in range(1, 8):
                if p + 1 < 8:
                    wts[p + 1] = dma_wt(p + 1)   # prefetch next pair
                kq_pair(p, wts.pop(p))
                es_of[2 * p] = scores_head(2 * p)
                es_of[2 * p + 1] = scores_head(2 * p + 1)
                if p == 1:
                    v_chunk(0, *wv0, tts=range(8, 16))
                    wv1 = dma_wv(1)
                pv_head(nexth, es_of.pop(nexth)); nexth += 1
                pv_head(nexth, es_of.pop(nexth)); nexth += 1
                if p in (2, 3, 4):               # spread V chunk 1
                    lo, hi = ((0, 6), (6, 11), (11, 16))[p - 2]
                    v_chunk(1, *wv1, tts=range(lo, hi))
                if p == 2:
                    nc.sync.dma_start(
                        out=projw_sb[:],
                        in_=projw_d[:].rearrange("(pr p) m -> p pr m",
                                                 p=128))
            # ---- projection (PSUM ring shared with the QKV phase) ----
            def proj_acc(ps, qt, n, prs, start):
                for pr in prs:
                    nc.tensor.matmul(
                        ps[:],
                        ctxpair_sb[:, pr, 128 * qt:128 * (qt + 1)],
                        projw_sb[:, pr, 512 * n:512 * (n + 1)],
                        start=(start and pr == prs[0]), stop=(pr == 7))

            def proj_fin(ps, qt, n):
                ot = npo.tile([128, 512], F32, tag="ot", bufs=3, name="ot")
                nc.scalar.copy(ot[:], ps[:])
                nc.sync.dma_start(
                    out=out_d[128 * qt:128 * (qt + 1),
                              512 * n:512 * (n + 1)],
                    in_=ot[:])

            while nexth < NH:
                pv_head(nexth, es_of.pop(nexth)); nexth += 1

            # stagger: the first two output tiles accumulate pairs 0..6
            # right after PV15 — that work runs concurrently with head
            # 15's normalize chain, so pr=7 (which needs it) never stalls
            psA = qps.tile([128, 512], F32, tag="qkvps", name="proj_psA")
            proj_acc(psA, 0, 0, list(range(7)), start=True)
            psB = qps.tile([128, 512], F32, tag="qkvps", name="proj_psB")
            proj_acc(psB, 0, 1, list(range(7)), start=True)
            proj_acc(psA, 0, 0, [7], start=False)
            proj_fin(psA, 0, 0)
            proj_acc(psB, 0, 1, [7], start=False)
            proj_fin(psB, 0, 1)
            for qt in range(1, 4):
                for n in range(2):
                    ps = qps.tile([128, 512], F32, tag="qkvps",
                                  name="proj_ps")
                    proj_acc(ps, qt, n, list(range(8)), start=True)
                    proj_fin(ps, qt, n)


# ---------------- host-side prep ----------------

def _make_rotary_map(sinusoids):
    sin = np.asarray(sinusoids[0], np.float32).T  # [ROT, S]
    cos = np.asarray(sinusoids[1], np.float32).T
    M = np.ones((DH, S), np.float32)
    sign = np.where(np.arange(ROT) % 2 == 0, -1.0, 1.0).astype(np.float32)
    M[:ROT] = cos + sign[:, None] * sin
    return M


def _host_prep(x, sinusoids, attention_bias, qkv_kernel, qkv_bias,
               proj_kernel):
    x = np.asarray(x, np.float32)
    sinusoids = np.asarray(sinusoids, np.float32)
    attention_bias = np.asarray(attention_bias, np.float32)
    qkv_kernel = np.asarray(qkv_kernel, np.float32)
    qkv_bias = np.asarray(qkv_bias, np.float32)
    proj_kernel = np.asarray(proj_kernel, np.float32)
    assert not np.any(qkv_bias), "nonzero qkv_bias not supported"

    M = _make_rotary_map(sinusoids)          # [64, S]
    scale = np.float32(1.0 / np.sqrt(DH))

    wqk = np.concatenate(
        [qkv_kernel[:, :NH, :].reshape(HID, HID),
         qkv_kernel[:, NH:2 * NH, :].reshape(HID, HID)], 1)
    wqk8 = np.ascontiguousarray(wqk * WSCALE).astype(f8)
    wvs = qkv_kernel[:, 2 * NH:, :].reshape(HID, HID) * WSCALE
    wv8 = wvs.astype(f8)
    rwv8 = np.ascontiguousarray(wvs - wv8.astype(np.float32)).astype(f8)
    wv8 = np.ascontiguousarray(wv8)
    projw = np.ascontiguousarray(proj_kernel.reshape(HID, HID)).astype(bf16)

    in_maps = []
    for i in range(N_CORES):
        b, r = i // 4, i % 4
        # rotate token axis so this core's queries are tokens [0, 512)
        perm = np.roll(np.arange(S), -SQ * r)
        xbT = np.ascontiguousarray(x[b][perm].T)             # [1024, S]
        xT8 = xbT.astype(f8)
        rxT8 = np.ascontiguousarray(
            xbT - xT8.astype(np.float32)).astype(f8)
        Mr = M[:, perm]
        mk = np.ascontiguousarray(np.tile(Mr / WSCALE, (2, 1))).astype(bf16)
        mq = np.ascontiguousarray(
            np.tile(Mr[:, :SQ] * scale / WSCALE, (2, 1))).astype(bf16)
        biasT = np.ascontiguousarray(
            np.exp(attention_bias[b, 0, SQ * r:SQ * (r + 1), :][:, perm].T)
        ).astype(bf16)
        in_maps.append({
            "xT": xT8, "rxT": rxT8, "wqk": wqk8, "wv": wv8, "rwv": rwv8,
            "biasT": biasT, "mq": mq, "mk": mk, "projw": projw,
        })
    return in_maps


def kernel(x, sinusoids, attention_bias, qkv_kernel, qkv_bias, proj_kernel,
           **_ignored):
    global _CACHED_NC
    if _CACHED_NC is None:
        _CACHED_NC = _build_nc()
    nc = _CACHED_NC

    in_maps = _host_prep(x, sinusoids, attention_bias, qkv_kernel,
                         qkv_bias, proj_kernel)
    trace = bool(os.environ.get("BASS_TRACE"))
    res = run_bass_kernel_spmd(nc, in_maps, core_ids=list(range(N_CORES)),
                               trace=trace)
    if res.exec_time_ns is not None:
        print(f"HW exec time: {res.exec_time_ns} ns")

    out = np.zeros((B, S, HID), np.float32)
    for i in range(N_CORES):
        b, r = i // 4, i % 4
        out[b, SQ * r:SQ * (r + 1), :] = res.results[i]["out"]
    return out


if __name__ == "__main__":
    rng = np.random.default_rng(0)
    ins = dict(
        x=rng.standard_normal((B, S, HID)).astype(np.float32),
        sinusoids=rng.uniform(-1, 1, (2, S, ROT)).astype(np.float32),
        attention_bias=(rng.standard_normal((B, 1, S, S)) * 0.1).astype(
            np.float32),
        qkv_kernel=(rng.standard_normal((HID, 48, DH)) * 0.0124).astype(
            np.float32),
        qkv_bias=np.zeros((48, DH), np.float32),
        proj_kernel=(rng.standard_normal((NH, DH, HID)) * 0.0124).astype(
            np.float32),
    )
    t0 = time.time()
    out = kernel(**ins)
    print(f"kernel() wall: {time.time()-t0:.1f}s out shape {out.shape}")

